# revision 24
# baseline (speedup 1.0000x reference)
"""Local (windowed) self-attention Trainium2 kernel.

Model (reference): LayerNorm -> per-window (W=1024) multi-head attention
(H=8 heads, K=32 head dim) -> output projection -> residual add.
Shapes: x [B=2, T=8192, C=512]; 16 independent windows of 1024 tokens.

Distribution: 16 windows / 8 cores = 2 windows per core (data parallel over
the B*n_chunks axis), QKV/O projection weights replicated, no collectives.

Per-core program (Tile framework, fully unrolled, bf16 matmuls / fp32 PSUM):
  Phase 1 (both windows): LayerNorm in [tok, C] layout via bn_stats,
    normalize + cast bf16, bounce z through DRAM and DMA-transpose back to
    zT [C, tok] (4 large [1024,128] transposes per window; small/SBUF-side
    transposes measured much slower on this machine).
  Phase 2 (per window):
    - QT, KT [hd, tok] projections (lhsT = folded weights); V [tok, hd]
      projection (lhsT = zT), stored augmented as [V_h | ones] 64-column
      blocks per head.
    - Attention pipelined over (q-tile 512, head pair, s-chunk 128):
      scoresT [s, q] per head via one matmul each (contraction = head dim
      32, row tile_position 32g) into a double-buffered 2-bank PSUM tile;
      ONE ScalarE Exp op [128, 1024] (scale=1/sqrt(K) folded; no max-
      subtraction needed, |scores| <~ 6) -> bf16 expT; then ONE M=64
      matmul per head (lhsT = [V_h | ones], col tile_position 0/64)
      accumulates BOTH attn@V (rows 0:32) and the softmax denominator
      (rows 32:64) over the 8 s-chunks. Scores for iteration i+1 are
      emitted before A*V of iteration i so the PE stream stays ahead of
      ScalarE (exp is ~1.1us/iter, PE ~1.1-1.4us/iter).
    - Normalize oT with VectorE reciprocal+multiply; output projection
      (contraction hd=256) + residual + bias adds on VectorE.

Host-side prep (constant folding only): shard windows, fold LN gamma/beta
into projection weights/biases, fold bv through attention (softmax rows sum
to 1) into the output bias, cast weights to bf16.

Measured (8 axon NeuronCores): ~280 us per invocation (R=128 vs R=16
hardware repeat-loop slope), output absmax relative error 1.4e-3 vs the
fp32 reference.
"""

import numpy as np
import ml_dtypes

import concourse.bass as bass
import concourse.tile as tile
from concourse import bacc, mybir
from concourse.bass_utils import run_bass_kernel_spmd

F32 = mybir.dt.float32
BF16 = mybir.dt.bfloat16

B, T, C, H, K = 2, 8192, 512, 8, 32
W = 1024
HK = H * K              # 256
N_CORES = 8
NW = (B * T) // W       # 16 windows
WPC = NW // N_CORES     # 2 windows per core
EPS = 1e-5
SCALE = 1.0 / np.sqrt(K)

TOK_TILES = W // 128    # 8 token tiles per window
C_CHUNKS = C // 128     # 4
HD_TILES = HK // 128    # 2
Q_TILES = W // 512      # 2 query tiles per window
S_CHUNKS = W // 128     # 8 key chunks per window
HPAIRS = H // 2         # 4 head pairs
EX_BUFS = 5
LN_BUFS = 8
ZW_BUFS = 6
OUTP_BUFS = 6


def _build_program(reps=1, do_ln=True, do_qkv=True, do_attn=True, do_av=True,
                   do_out=True, do_exp=True, ln_dram_bounce=True, alt_hp=False,
                   ln_fine_transpose=False, do_inject=False):
    nc = bacc.Bacc("TRN2", target_bir_lowering=False, debug=False)

    x_d = nc.dram_tensor("x", [WPC * W, C], F32, kind="ExternalInput")
    wq_d = nc.dram_tensor("wq", [C_CHUNKS, 128, HK], BF16, kind="ExternalInput")
    wk_d = nc.dram_tensor("wk", [C_CHUNKS, 128, HK], BF16, kind="ExternalInput")
    wv_d = nc.dram_tensor("wv", [C_CHUNKS, 128, HK], BF16, kind="ExternalInput")
    wo_d = nc.dram_tensor("wo", [HD_TILES, 128, C], BF16, kind="ExternalInput")
    bq_d = nc.dram_tensor("bq", [HD_TILES, 128, 1], F32, kind="ExternalInput")
    bk_d = nc.dram_tensor("bk", [HD_TILES, 128, 1], F32, kind="ExternalInput")
    bo_d = nc.dram_tensor("bo", [1, C], F32, kind="ExternalInput")
    out_d = nc.dram_tensor("out", [WPC * W, C], F32, kind="ExternalOutput")
    z_d = nc.dram_tensor("z_scratch", [WPC, W, C], BF16)

    with tile.TileContext(nc) as tc:
        with (
            tc.tile_pool(name="const", bufs=1) as const,
            tc.tile_pool(name="xres", bufs=1) as xres,
            tc.tile_pool(name="zt", bufs=1) as ztp,
            tc.tile_pool(name="ln", bufs=LN_BUFS) as ln,
            tc.tile_pool(name="zw", bufs=ZW_BUFS) as zw,
            tc.tile_pool(name="qk", bufs=2) as qk,
            tc.tile_pool(name="vp", bufs=2) as vp,
            tc.tile_pool(name="ot", bufs=2) as otp,
            tc.tile_pool(name="ex", bufs=EX_BUFS) as ex,
            tc.tile_pool(name="tmp", bufs=8) as tmp,
            tc.tile_pool(name="outp", bufs=OUTP_BUFS) as outp,
            tc.tile_pool(name="ps_proj", bufs=2, space="PSUM") as ps_proj,
            tc.tile_pool(name="ps_sc", bufs=2, space="PSUM") as ps_sc_pool,
            tc.tile_pool(name="ps_acc", bufs=2, space="PSUM") as ps_acc,
        ):
            from contextlib import ExitStack as _ES
            _es = _ES()
            if reps > 1:
                _es.enter_context(
                    tc.For_i(
                        0, reps, 1,
                        hint_engines=(
                            mybir.EngineType.PE,
                            mybir.EngineType.Activation,
                            mybir.EngineType.DVE,
                            mybir.EngineType.SP,
                        ),
                    )
                )
            # ---- constants / weights -------------------------------------
            eps_t = const.tile([128, 1], F32)
            nc.vector.memset(eps_t, EPS)

            wq_s = const.tile([128, C_CHUNKS, HK], BF16)
            wk_s = const.tile([128, C_CHUNKS, HK], BF16)
            wv_s = const.tile([128, C_CHUNKS, HK], BF16)
            for c in range(C_CHUNKS):
                nc.sync.dma_start(wq_s[:, c, :], wq_d[c])
                nc.sync.dma_start(wk_s[:, c, :], wk_d[c])
                nc.sync.dma_start(wv_s[:, c, :], wv_d[c])
            wo_s = const.tile([128, HD_TILES, C], BF16)
            for g in range(HD_TILES):
                nc.sync.dma_start(wo_s[:, g, :], wo_d[g])
            bq_s = const.tile([128, HD_TILES], F32)
            bk_s = const.tile([128, HD_TILES], F32)
            for m in range(HD_TILES):
                nc.sync.dma_start(bq_s[:, m : m + 1], bq_d[m])
                nc.sync.dma_start(bk_s[:, m : m + 1], bk_d[m])
            # bo broadcast to all 128 partitions (DMA with 0-stride source)
            bo_s = const.tile([128, C], F32)
            bo_bcast_ap = bass.AP(
                tensor=bo_d.ap().tensor,
                offset=0,
                ap=[[0, 128], [1, C]],
            )
            nc.sync.dma_start(bo_s, bo_bcast_ap)

            # ---- phase 1: LayerNorm + transpose (both windows) -----------
            xs = [
                [xres.tile([128, C], F32, name=f"x_{w}_{t}", tag=f"x_{w}_{t}")
                 for t in range(TOK_TILES)]
                for w in range(WPC)
            ]
            zT = [
                [ztp.tile([128, W], BF16, name=f"zT_{w}_{c}", tag=f"zT_{w}_{c}")
                 for c in range(C_CHUNKS)]
                for w in range(WPC)
            ]
            if not do_ln:
                for w in range(WPC):
                    for t in range(TOK_TILES):
                        nc.sync.dma_start(
                            xs[w][t], x_d[(w * TOK_TILES + t) * 128 :][:128, :]
                        )
                    for c in range(C_CHUNKS):
                        nc.gpsimd.memset(zT[w][c], 0.001)
            for w in range(WPC if do_ln else 0):
                for t in range(TOK_TILES):
                    x_t = xs[w][t]
                    nc.sync.dma_start(x_t, x_d[(w * TOK_TILES + t) * 128 :][:128, :])
                    stats = ln.tile([128, 6], F32, tag="stats")
                    nc.vector.bn_stats(out=stats, in_=x_t)
                    mv = ln.tile([128, 2], F32, tag="mv")
                    nc.vector.bn_aggr(out=mv, in_=stats)
                    std = ln.tile([128, 1], F32, tag="std")
                    nc.scalar.activation(
                        out=std,
                        in_=mv[:, 1:2],
                        func=mybir.ActivationFunctionType.Sqrt,
                        bias=eps_t[:],
                    )
                    rstd = ln.tile([128, 1], F32, tag="rstd")
                    nc.vector.reciprocal(out=rstd, in_=std)
                    z_t = zw.tile([128, C], BF16, tag="z")
                    nc.vector.tensor_scalar(
                        out=z_t,
                        in0=x_t,
                        scalar1=mv[:, 0:1],
                        scalar2=rstd,
                        op0=mybir.AluOpType.subtract,
                        op1=mybir.AluOpType.mult,
                    )
                    if ln_dram_bounce:
                        nc.sync.dma_start(z_d[w, t * 128 :][:128, :], z_t)
                        if ln_fine_transpose:
                            for c in range(C_CHUNKS):
                                nc.sync.dma_start(
                                    zT[w][c][:, t * 128 : (t + 1) * 128],
                                    z_d[w][t * 128 : (t + 1) * 128,
                                           c * 128 : (c + 1) * 128],
                                    transpose=True,
                                )
                    else:
                        for c in range(C_CHUNKS):
                            nc.sync.dma_start(
                                zT[w][c][:, t * 128 : (t + 1) * 128],
                                z_t[:, c * 128 : (c + 1) * 128],
                                transpose=True,
                            )
                if ln_dram_bounce and not ln_fine_transpose:
                    for c in range(C_CHUNKS):
                        nc.sync.dma_start(
                            zT[w][c], z_d[w][:, c * 128 : (c + 1) * 128],
                            transpose=True,
                        )

            # ---- phase 2: both windows, proj work injected into attention ---
            # Allocate per-window destination tiles eagerly (slot assignment
            # only; writes are emitted later by thunks).
            qkt = {}
            vs = {}
            oTs = {}
            for w in range(WPC):
                for name in ("q", "k"):
                    for m in range(HD_TILES):
                        qkt[(w, name, m)] = qk.tile(
                            [128, W], BF16,
                            name=f"{name}T_{w}_{m}", tag=f"{name}T_{m}",
                        )
                v_s = vp.tile([128, TOK_TILES, H, 64], BF16, name=f"v_{w}", tag="v")
                vs[w] = v_s
                nc.vector.memset(v_s[:, :, :, 32:64], 1.0)
                oTs[w] = [
                    otp.tile([128, W], BF16, name=f"oT_{w}_{g}", tag=f"oT_{g}")
                    for g in range(HD_TILES)
                ]

            def qkv_thunks(w):
                """One thunk per PSUM group of the QKV projections."""
                if not do_qkv:
                    def th_init():
                        for name in ("q", "k"):
                            for m in range(HD_TILES):
                                nc.gpsimd.memset(qkt[(w, name, m)], 0.001)
                        nc.gpsimd.memset(vs[w][:, :, :, 0:32], 0.001)
                    return [th_init]
                ths = []
                for name, w_s, b_s in (("q", wq_s, bq_s), ("k", wk_s, bk_s)):
                    for m in range(HD_TILES):
                        for n in range(Q_TILES):
                            def th(name=name, w_s=w_s, b_s=b_s, m=m, n=n):
                                dst = qkt[(w, name, m)]
                                ps = ps_proj.tile(
                                    [128, 512], F32, name="ps_p", tag="proj"
                                )
                                for c in range(C_CHUNKS):
                                    nc.tensor.matmul(
                                        ps,
                                        lhsT=w_s[:, c, m * 128 : (m + 1) * 128],
                                        rhs=zT[w][c][:, n * 512 : (n + 1) * 512],
                                        start=(c == 0),
                                        stop=(c == C_CHUNKS - 1),
                                    )
                                nc.vector.tensor_scalar_add(
                                    out=dst[:, n * 512 : (n + 1) * 512],
                                    in0=ps,
                                    scalar1=b_s[:, m : m + 1],
                                )
                            ths.append(th)
                for t in range(TOK_TILES):
                    def th(t=t):
                        ps = ps_proj.tile([128, 512], F32, name="ps_p", tag="proj")
                        psv = ps[:, :HK]
                        for c in range(C_CHUNKS):
                            nc.tensor.matmul(
                                psv,
                                lhsT=zT[w][c][:, t * 128 : (t + 1) * 128],
                                rhs=wv_s[:, c, :],
                                start=(c == 0),
                                stop=(c == C_CHUNKS - 1),
                            )
                        nc.vector.tensor_copy(
                            vs[w][:, t, :, 0:32],
                            psv.rearrange("p (h k) -> p h k", h=H),
                        )
                    ths.append(th)
                return ths

            def outproj_thunk(w, t):
                def th():
                    oT = oTs[w]
                    ps = ps_proj.tile([128, 512], F32, name="ps_p", tag="proj")
                    for g in range(HD_TILES):
                        nc.tensor.matmul(
                            ps,
                            lhsT=oT[g][:, t * 128 : (t + 1) * 128],
                            rhs=wo_s[:, g, :],
                            start=(g == 0),
                            stop=(g == HD_TILES - 1),
                        )
                    o_t = outp.tile([128, C], F32, tag="o")
                    nc.vector.tensor_add(out=o_t, in0=ps, in1=xs[w][t])
                    nc.vector.tensor_add(out=o_t, in0=o_t, in1=bo_s)
                    nc.sync.dma_start(
                        out_d[(w * TOK_TILES + t) * 128 :][:128, :], o_t
                    )
                return th

            def attn_emit(w, inject):
                """inject: dict {iter_index: [thunks]} emitted inside the
                pipeline (between the lookahead scores and this iteration's
                A*V) to fill PE gaps."""
                oT = oTs[w]
                if not (do_attn and do_av):
                    for g in range(HD_TILES):
                        nc.gpsimd.memset(oT[g], 0.001)
                if not do_attn:
                    for i in sorted(inject):
                        for th in inject[i]:
                            th()
                    return
                # head pairs alternate (hp even/odd) so consecutive
                # iterations touch different PE row halves (LDW pull-ahead).
                if alt_hp:
                    iters = [
                        (qt, 2 * hpp + sub, cch)
                        for qt in range(Q_TILES)
                        for hpp in range(HPAIRS // 2)
                        for cch in range(S_CHUNKS)
                        for sub in range(2)
                    ]
                else:
                    iters = [
                        (qt, hp, cch)
                        for qt in range(Q_TILES)
                        for hp in range(HPAIRS)
                        for cch in range(S_CHUNKS)
                    ]

                def emit_scores(qt, hp, cch):
                    ps_sc = ps_sc_pool.tile([128, 1024], F32, name="ps_sc", tag="sc")
                    hg = hp // 2
                    for j in range(2):
                        h = 2 * hp + j
                        g = h - 4 * hg
                        nc.tensor.matmul(
                            ps_sc[:, j * 512 : (j + 1) * 512],
                            lhsT=qkt[(w, "k", hg)][
                                g * 32 : (g + 1) * 32,
                                cch * 128 : (cch + 1) * 128,
                            ],
                            rhs=qkt[(w, "q", hg)][
                                g * 32 : (g + 1) * 32,
                                qt * 512 : (qt + 1) * 512,
                            ],
                            tile_position=(g * 32, 0),
                        )
                    return ps_sc

                accs = {}
                ps_sc = emit_scores(*iters[0])
                for i, (qt, hp, cch) in enumerate(iters):
                    if cch == 0:
                        accs[hp] = ps_acc.tile([128, 512], F32, name="ps_av", tag="acc")
                    acc = accs[hp]
                    expT = ex.tile([128, 1024], BF16, name="expT", tag="exp")
                    if do_exp:
                        nc.scalar.activation(
                            out=expT, in_=ps_sc,
                            func=mybir.ActivationFunctionType.Exp,
                            scale=float(SCALE),
                        )
                    elif do_av:
                        nc.vector.memset(expT, 0.001)
                    # next iteration's scores BEFORE this iteration's AV
                    if i + 1 < len(iters):
                        ps_sc = emit_scores(*iters[i + 1])
                    for th in inject.get(i, ()):
                        th()
                    for j in range(2 if do_av else 0):
                        h = 2 * hp + j
                        nc.tensor.matmul(
                            acc[j * 64 : (j + 1) * 64, :],
                            lhsT=vs[w][:, cch, h, :],
                            rhs=expT[:, j * 512 : (j + 1) * 512],
                            start=(cch == 0),
                            stop=(cch == S_CHUNKS - 1),
                            tile_position=(0, j * 64),
                            skip_group_check=True,
                        )
                    if do_av and cch == S_CHUNKS - 1:
                        for j in range(2):
                            h = 2 * hp + j
                            hg = h // 4
                            g = h - 4 * hg
                            rec = tmp.tile([32, 512], F32, tag=f"rec{j}")
                            nc.vector.reciprocal(
                                out=rec, in_=acc[j * 64 + 32 : j * 64 + 64, :]
                            )
                            nc.vector.tensor_mul(
                                out=oT[hg][
                                    g * 32 : (g + 1) * 32,
                                    qt * 512 : (qt + 1) * 512,
                                ],
                                in0=acc[j * 64 : j * 64 + 32, :],
                                in1=rec,
                            )
                for i in sorted(inject):
                    if i >= len(iters):
                        for th in inject[i]:
                            th()

            n_iters = Q_TILES * HPAIRS * S_CHUNKS

            # window 0: QKV up front (nothing to hide it under)
            for th in qkv_thunks(0):
                th()
            # attention(w0) with QKV(w1) injected
            inj0 = {}
            ths_w1 = qkv_thunks(1)
            if do_inject:
                step = max(1, n_iters // (len(ths_w1) + 1))
                for idx, th in enumerate(ths_w1):
                    inj0.setdefault(min((idx + 1) * step, n_iters - 1), []).append(th)
                attn_emit(0, inj0)
            else:
                attn_emit(0, {})
                for th in ths_w1:
                    th()

            # attention(w1) with outproj(w0) injected early and
            # outproj(w1) for qt=0 tiles injected in the qt=1 half
            inj1 = {}
            if do_out and do_inject:
                for idx, t in enumerate(range(TOK_TILES)):
                    inj1.setdefault(min(4 + idx * 7, n_iters - 1), []).append(
                        outproj_thunk(0, t)
                    )
                half = n_iters // 2
                for idx, t in enumerate(range(TOK_TILES // 2)):
                    inj1.setdefault(
                        min(half + 4 + idx * 10, n_iters - 1), []
                    ).append(outproj_thunk(1, t))
                for idx, t in enumerate(range(TOK_TILES // 2, TOK_TILES)):
                    inj1.setdefault(n_iters + idx, []).append(outproj_thunk(1, t))
                attn_emit(1, inj1)
            else:
                if do_out:
                    for t in range(TOK_TILES):
                        outproj_thunk(0, t)()
                attn_emit(1, {})
                if do_out:
                    for t in range(TOK_TILES):
                        outproj_thunk(1, t)()

            _es.close()

    nc.compile()
    return nc


_CACHE = {}


def _get_program():
    if "nc" not in _CACHE:
        _CACHE["nc"] = _build_program()
    return _CACHE["nc"]


def _prep_inputs(x, ln_gamma, ln_beta, Wq, bq, Wk, bk, Wv, bv, Wo, bo):
    """Host-side constant folding + sharding. Returns per-core in_maps."""
    x = np.asarray(x, np.float32)
    g = np.asarray(ln_gamma, np.float32)
    be = np.asarray(ln_beta, np.float32)
    Wq = np.asarray(Wq, np.float32).reshape(C, HK)
    Wk = np.asarray(Wk, np.float32).reshape(C, HK)
    Wv = np.asarray(Wv, np.float32).reshape(C, HK)
    Wo2 = np.asarray(Wo, np.float32).reshape(HK, C)
    bq = np.asarray(bq, np.float32).reshape(HK)
    bk = np.asarray(bk, np.float32).reshape(HK)
    bv = np.asarray(bv, np.float32).reshape(HK)
    bo = np.asarray(bo, np.float32).reshape(C)

    # Fold LN affine (z = n*gamma + beta) into projections:
    #   z @ W + b = n @ (gamma[:,None]*W) + (beta @ W + b)
    Wq_e = g[:, None] * Wq
    Wk_e = g[:, None] * Wk
    Wv_e = g[:, None] * Wv
    bq_e = be @ Wq + bq
    bk_e = be @ Wk + bk
    bv_e = be @ Wv + bv
    # Softmax rows sum to 1 -> value bias passes through attention:
    #   attn @ (V + 1 bv) @ Wo + bo = attn @ V @ Wo + (bv @ Wo + bo)
    bo_e = bv_e @ Wo2 + bo

    bf = ml_dtypes.bfloat16
    wq_h = Wq_e.reshape(C_CHUNKS, 128, HK).astype(bf)
    wk_h = Wk_e.reshape(C_CHUNKS, 128, HK).astype(bf)
    wv_h = Wv_e.reshape(C_CHUNKS, 128, HK).astype(bf)
    wo_h = Wo2.reshape(HD_TILES, 128, C).astype(bf)
    bq_h = bq_e.reshape(HD_TILES, 128, 1).astype(np.float32)
    bk_h = bk_e.reshape(HD_TILES, 128, 1).astype(np.float32)
    bo_h = bo_e.reshape(1, C).astype(np.float32)

    xw = np.ascontiguousarray(x.reshape(NW, W, C))
    in_maps = []
    for i in range(N_CORES):
        shard = np.ascontiguousarray(
            xw[i * WPC : (i + 1) * WPC].reshape(WPC * W, C)
        )
        in_maps.append(
            {
                "x": shard,
                "wq": wq_h, "wk": wk_h, "wv": wv_h, "wo": wo_h,
                "bq": bq_h, "bk": bk_h, "bo": bo_h,
            }
        )
    return in_maps


def kernel(x, ln_gamma, ln_beta, Wq, bq, Wk, bk, Wv, bv, Wo, bo):
    nc = _get_program()
    in_maps = _prep_inputs(x, ln_gamma, ln_beta, Wq, bq, Wk, bk, Wv, bv, Wo, bo)
    res = run_bass_kernel_spmd(nc, in_maps, core_ids=list(range(N_CORES)))
    out = np.concatenate([res.results[i]["out"] for i in range(N_CORES)], axis=0)
    return np.ascontiguousarray(out.reshape(B, T, C)).astype(np.float32)
